# revision 3
# baseline (speedup 1.0000x reference)
"""Fused ConvTranspose3d(stride2,pad1) + scale + AvgPool3d(2) + bias kernel for TRN2.

Math: transposed conv (K=3,S=2,P=1) + AvgPool(2) collapse into a stride-1 VALID
conv with a 2x2x2 kernel: per-dim taps S0 = W[1]+W[2] (pairs x[o]), S1 = W[0]
(pairs x[o+1]); z = conv(x, V)*(s1*s2/8) + beta, beta = (conv_bias*s1+bias)*s2.

Mapping (w-stagger):
  k = 128 = (b:2 h-taps, a:2 d-taps, ci:32); x4 SBUF tile holds 4 shifted
      replicas of x built on-chip: rows[32:64] = rows[0:32]+PLANE (DVE copy),
      rows[64:128] = rows[0:64]+W (DVE copy). Shifts are 4B-aligned -> DVE 4x.
  m = 128 = (s:2 w-parity, co:64); psum[(s,co), (r, j)] = z[co, od, r, 2j+s].
  3 accumulating matmuls per (slab, bank), phase p reads rhs at col offset +p
  with stride-2 w so each column feeds both w-parities:
      W0 = [U0 | 0], W1 = [U1 | U0], W2 = [0 | U1]  (column halves = s).
  Tail: single psum->SBUF bf16 copy per slab (ACT/Pool rotation), bias+f32 on host.
Data parallel: batch 16 -> 2 per core on 8 cores. All HBM I/O in bf16.
"""

import sys

if "/opt/trn_rl_repo" not in sys.path:
    sys.path.insert(0, "/opt/trn_rl_repo")

from contextlib import ExitStack

import numpy as np
import ml_dtypes

import concourse.bass as bass
import concourse.tile as tile
from concourse import mybir
from concourse.bass_utils import run_bass_kernel_spmd
from concourse.vector_clock import ScopedClock as _ScopedClock

BF16 = ml_dtypes.bfloat16


# walrus codegen allows only one sync-wait per TPB_CTRL instruction; split the
# Tile tail-drain's waits across single-wait nop carriers.
def _patched_drain_and_barrier(self, tick_clock, wait_clock):
    nc = self.nc
    drain_inst = nc.sync.drain()
    wait_clock.add_sem_waits(
        drain_inst.ins, _ScopedClock({None: tick_clock.global_clock})
    )
    waits = list(drain_inst.ins.sync_info.on_wait)
    if len(waits) > 1:
        drain_inst.ins.sync_info.on_wait = waits[:1]
        for w in waits[1:]:
            n = nc.sync.nop(nofuse=True)
            n.ins.sync_info = mybir.SyncInfo(on_wait=[w], on_update=[])
    nc.all_engine_barrier()
    assert self.sems is not None
    popped = nc._tile_sem_poison_stack.pop()
    assert popped is self._sem_poison
    nc.clear_and_free_semaphores(list(self.sems.allocated().values()))
    nc.all_engine_barrier()


tile.TileContext._drain_and_barrier = _patched_drain_and_barrier


def _legalize_sync_waits(nc, max_waits=1):
    """walrus codegen allows very few sync-waits per instruction; move excess
    waits onto nop carriers on the same engine right before the instruction."""
    for fn in nc.m.functions:
        for bb in fn.blocks:
            new_insts = []
            changed = False
            for inst in bb.instructions:
                si = getattr(inst, "sync_info", None)
                if si is not None and si.on_wait and len(si.on_wait) > max_waits:
                    waits = list(si.on_wait)
                    si.on_wait = waits[-max_waits:]
                    extra = waits[:-max_waits]
                    for i in range(0, len(extra), max_waits):
                        nop = mybir.InstNoOp(
                            name=nc.get_next_instruction_name(),
                            engine=inst.engine,
                            sync_info=mybir.SyncInfo(
                                on_wait=extra[i : i + max_waits], on_update=[]
                            ),
                            bass_nofuse=True,
                        )
                        new_insts.append(nop)
                    changed = True
                new_insts.append(inst)
            if changed:
                bb.instructions[:] = new_insts


N, C_IN, C_OUT = 16, 32, 64
D = H = W = 32
OD = OH = OW = 31
NCORES = 8
NB = N // NCORES
PLANE = H * W  # 1024
VOL = D * PLANE
NJ = 16  # w-pair columns per row
ZR = 31  # valid output rows per slab

_CHUNKS = [(0, 1), (1, 7), (8, 8), (16, 8), (24, 4), (28, 3)]


def _build_program(chunks=_CHUNKS, nb=NB, legalize=True, guard_memset=False):
    nc = bass.Bass(
        "TRN2", target_bir_lowering=False, debug=False, num_swdge_queues=4
    )
    f32 = mybir.dt.float32
    bf16 = mybir.dt.bfloat16
    x_ap = nc.dram_tensor("x", [nb, C_IN, VOL], bf16, kind="ExternalInput").ap()
    w_ap = nc.dram_tensor("wu", [128, 2, C_OUT], bf16, kind="ExternalInput").ap()
    z_ap = nc.dram_tensor(
        "z", [nb, 2, C_OUT, OD, ZR, NJ], bf16, kind="ExternalOutput"
    ).ap()

    with tile.TileContext(nc) as tc, ExitStack() as ctx:
        wpool = ctx.enter_context(tc.tile_pool(name="w", bufs=1))
        x4pool = ctx.enter_context(tc.tile_pool(name="x4", bufs=4))
        pspool = ctx.enter_context(tc.tile_pool(name="ps", bufs=4, space="PSUM"))
        ogpool = ctx.enter_context(tc.tile_pool(name="og", bufs=6))

        wt = wpool.tile([128, 2, C_OUT], bf16)
        nc.sync.dma_start(wt[:], w_ap[:])

        _dma_rr = [0]
        # interleave batches so batch transitions pipeline like any other
        # chunk transition
        sched = [
            (b, od0, nsl) for od0, nsl in chunks for b in range(nb)
        ]
        if True:
            for b, od0, nsl in sched:
                npl = min(nsl + 1, D - od0)  # planes loaded
                ext_load = npl * PLANE
                rep1 = nsl * PLANE + 2 * W + 4  # rows[32:64] extent
                rep2 = nsl * PLANE + W + 2  # rows[64:128] extent
                ext = ext_load + 2 * W + 4  # tile extent incl. guard
                x4 = x4pool.tile([128, ext], bf16, tag="x4")
                # deterministic guard so replica copies read defined data
                if guard_memset:
                    # sim-only: keep the race detector happy about guard reads
                    # (they only ever feed the never-stored w=31 column)
                    nc.vector.memset(x4[0:32, ext_load:ext], 0.0)
                # loads on the sync queue only: stores live on gpsimd so a
                # store waiting for evacs never blocks the next chunk's load
                nc.sync.dma_start(
                    x4[0:32, 0:ext_load],
                    x_ap[b, :, od0 * PLANE : od0 * PLANE + ext_load],
                )

                # replica pieces: matmuls of each slab group start as soon as
                # its piece of the replicas lands; later pieces overlap the
                # matmuls on DVE. rows[32:64] = +PLANE; rows[64:128] = +W.
                if nsl >= 6:
                    bnds = [0, (nsl + 2) // 3, (2 * nsl + 2) // 3, nsl]
                elif nsl >= 3:
                    bnds = [0, (nsl + 1) // 2, nsl]
                else:
                    bnds = [0, nsl]
                groups = [
                    list(range(bnds[k], bnds[k + 1]))
                    for k in range(len(bnds) - 1)
                ]
                prev1 = prev2 = 0
                piece_ends = []
                for k in range(1, len(bnds)):
                    last = bnds[k] == nsl
                    a1 = rep1 if last else min((bnds[k] + 1) * PLANE, rep1)
                    a2 = rep2 if last else min(bnds[k] * PLANE + W + 2, rep2)
                    nc.vector.tensor_copy(
                        x4[32:64, prev1:a1],
                        x4[0:32, PLANE + prev1 : PLANE + a1],
                    )
                    nc.vector.tensor_copy(
                        x4[64:128, prev2:a2], x4[0:64, W + prev2 : W + a2]
                    )
                    prev1, prev2 = a1, a2
                    piece_ends.append((a1, a2))

                ntile = (nsl + 1) // 2
                pss = []
                for t in range(ntile):
                    nsl_t = min(2, nsl - 2 * t)
                    ps = pspool.tile(
                        [128, 2, 512], f32, tag="ps", name=f"ps{t}"
                    )
                    pss.append((ps, nsl_t))
                # column-tiled matmuls: the two w-parity halves (s) run as
                # concurrent m=64 col-groups of the PE array; each does its
                # own 2-tap (U0, U1) psum accumulation -> no zero-weight waste
                for grp in groups:
                    for c in range(2):
                        lhsT = wt[:, c, :]
                        for i in grp:
                            ps = pss[i // 2][0]
                            for s in range(2):
                                base = i * PLANE + s + c
                                rhs = x4[:, base : base + PLANE].rearrange(
                                    "k (r w) -> k r w", w=W
                                )[:, :, 0:32:2][:, 0:ZR]
                                nc.tensor.matmul(
                                    ps[
                                        64 * s : 64 * s + 64,
                                        i % 2,
                                        0 : ZR * NJ,
                                    ].rearrange("m (r j) -> m r j", j=NJ),
                                    lhsT,
                                    rhs,
                                    start=(c == 0),
                                    stop=(c == 1),
                                    skip_group_check=True,
                                )
                # evac per psum tile (ACT only); one store per chunk
                zmerged = z_ap[b].rearrange("s co od r j -> (s co) od (r j)")
                og = ogpool.tile(
                    [128, nsl, ZR, NJ], bf16, tag="og", name="og"
                )
                off = 0
                for ps, nsl_t in pss:
                    src = ps[:, 0:nsl_t, 0 : ZR * NJ].rearrange(
                        "m s (r j) -> m s r j", j=NJ
                    )
                    nc.scalar.copy(og[:, off : off + nsl_t], src)
                    off += nsl_t
                zdst = zmerged[:, od0 : od0 + nsl, :]
                nc.gpsimd.dma_start(
                    zdst, og[:].rearrange("p i r j -> p i (r j)")
                )
    if legalize:
        _legalize_sync_waits(nc)
    return nc


def _host_prep(weight, conv_bias, bias, scale1, scale2):
    w = np.asarray(weight, dtype=np.float64)  # (C_IN, C_OUT, 3, 3, 3)
    s1 = float(np.asarray(scale1))
    s2 = float(np.asarray(scale2))
    alpha = s1 * s2 / 8.0
    taps = [[1, 2], [0]]  # S0 = W[1]+W[2] pairs x[o]; S1 = W[0] pairs x[o+1]
    # U[c][(b,a,ci), co]
    U = np.zeros((2, 128, C_OUT), dtype=np.float64)
    for c in range(2):
        for b_ in range(2):
            for a in range(2):
                v = np.zeros((C_IN, C_OUT), dtype=np.float64)
                for kd in taps[a]:
                    for kh in taps[b_]:
                        for kw in taps[c]:
                            v += w[:, :, kd, kh, kw]
                r0 = b_ * 64 + a * 32
                U[c, r0 : r0 + 32, :] = alpha * v
    wm = np.zeros((128, 2, C_OUT), dtype=np.float32)
    wm[:, 0, :] = U[0]
    wm[:, 1, :] = U[1]
    beta = (
        (np.asarray(conv_bias, np.float64).reshape(-1) * s1
         + np.asarray(bias, np.float64).reshape(-1)) * s2
    ).astype(np.float32)
    return wm.astype(BF16), beta


def _assemble(z_raw, beta):
    """z_raw [nb, 2, 64, OD, ZR, NJ] bf16 -> [nb, 64, OD, OH, OW] f32 + beta."""
    zc = np.moveaxis(np.asarray(z_raw, dtype=np.float32), 1, -1)
    # [nb, 64, OD, ZR, NJ, 2] -> w = 2j+s
    nb = zc.shape[0]
    zc = zc.reshape(nb, C_OUT, OD, ZR, NJ * 2)[..., :OW]
    return zc + beta.reshape(1, C_OUT, 1, 1, 1)


def kernel(x, weight, conv_bias, bias, scale1, scale2, _trace=False):
    x_bf = (
        np.asarray(x, dtype=np.float32)
        .reshape(N, C_IN, VOL)
        .astype(BF16)
    )
    wm, beta = _host_prep(weight, conv_bias, bias, scale1, scale2)

    nc = _build_program()
    in_maps = []
    for core in range(NCORES):
        in_maps.append(
            {
                "x": np.ascontiguousarray(x_bf[core * NB : (core + 1) * NB]),
                "wu": wm,
            }
        )
    res = run_bass_kernel_spmd(
        nc, in_maps, core_ids=list(range(NCORES)), trace=_trace
    )
    z = np.empty((N, C_OUT, OD, OH, OW), dtype=np.float32)
    for core in range(NCORES):
        z[core * NB : (core + 1) * NB] = _assemble(res.results[core]["z"], beta)
    if _trace:
        return z, res
    return z
